# revision 27
# baseline (speedup 1.0000x reference)
"""Trainium2 Bass kernel for AlignOnlySubLayer.

Per batch b:
    W[c,m]   = sum_d context[b,c,d] * main[b,m,d]
    A        = softmax(W, axis=m)
    out[m,d] = main[b,m,d] - sum_c A[c,m] * context[b,c,d]

Sharding: data-parallel over batch B=8 across the 8 NeuronCores (one batch
per core, no cross-core communication).

Kernel design (per core):
  - ACT is the algorithmic wall: 4M exps at 1 elem/lane/cycle @1.2GHz ~= 2
    x 1.1us half-row ACTIVATEs per c-tile.  Everything else is scheduled
    to keep that chain dense from ~5us after kernel start to the end.
  - Linear DMA tiling: c/m tile j = rows {16p + j} (partition p = row//16)
    instead of row%128.  Per-partition DMA runs are then 8KB contiguous
    (vs 512B for the row%128 tiling), so the 2MB of loads and 1MB of
    stores run near DMA line rate.  All compute is tile-local, so this is
    a pure relabeling - softmax rows stay intact, mm2's m-blocks match
    main's layout, and load/store/compute all agree.
  - Loads are split across both HWDGE rings + SWDGE: sync carries ctx q0
    / main q0 / main q1 (what the first c-tile needs), gpsimd carries the
    rest.  ACT's queue carries no DMA issues at all before the exp chain.
  - Both matmuls run fp16/bf16 (PE 1 col/cycle) with f32 PSUM.  mm1 fp16
    (mantissa), mm2 bf16 (E reaches ~e^70, needs f32 exponent range).
  - d-major operands: ctx q0 + main q0 transposed on PE (transpose-mode
    matmuls staged through a psum_w slot, DVE evacuation); main q1-3 and
    ctx q1-3 ride the DMA xbar, which is otherwise idle after the loads.
  - acc is initialized with +main via a PE identity-matmul pass (start=
    True), and the softmax normalization is folded into ctx with a
    NEGATED scale (ctx_s = -context/S), so acc accumulates main-weighted
    directly: the tail needs only a PSUM->SBUF copy (2x rate) instead of
    a tensor_tensor subtract (1x), and no separate subtract pass.
  - Software pipeline, one c-tile lookahead: iteration ct emits
    exp(ct)h0, mm2(ct-1)[0..7], mm1(ct+1)h0, exp(ct)h1+accum,
    mm2(ct-1)[8..15], mm1(ct+1)h1, stats(ct).  PE per-period work
    (~2.2us) phases into ACT's ~2.3us period without stalling the exps;
    the two psum_w slots alternate h0/h1 so mm1(ct+1)h0 only waits on
    exp(ct)h0.
  - Row sums: h0 reduced on DVE (off ACT's critical path), h1 via ACT's
    fused accum_out; s = -(h0+h1) and reciprocal give the negated scale.
"""

import numpy as np

import concourse.bass as bass
import concourse.mybir as mybir
from concourse import bacc
from concourse.masks import make_identity
from concourse.tile import TileContext

P = 128
F32 = mybir.dt.float32
F16 = mybir.dt.float16
BF16 = mybir.dt.bfloat16
EXP = mybir.ActivationFunctionType.Exp
AX = mybir.AxisListType.X
ADD = mybir.AluOpType.add
MULT = mybir.AluOpType.mult
N_CORES = 8


def build_nc(S=2048, D=128, num_devices=N_CORES, repeats=1, precise=False):
    """Build the single-core Bass program (SPMD across cores)."""
    assert D == P and S % P == 0
    T = S // P            # tiles along c (and m); tile j = rows {T*p + j}
    NQ = 4                # quarters (load/store/transpose granularity)
    QT = T // NQ          # tiles per quarter
    QW = QT * P           # f32 elements per partition per quarter
    HALF = S // 2         # columns per mm1 psum half
    MMN = 512             # mm1 moving-operand chunk

    nc = bacc.Bacc(
        "TRN2",
        target_bir_lowering=False,
        debug=False,
        enable_asserts=False,
        num_devices=num_devices,
    )
    ctx_d = nc.dram_tensor("context", [S, D], F32, kind="ExternalInput").ap()
    main_d = nc.dram_tensor("main", [S, D], F32, kind="ExternalInput").ap()
    # bf16 output (cast to f32 on host): halves store bytes+descriptors,
    # costs ~2e-3 relative rounding on top of the ~1.8e-3 fp16-pipeline
    # error -- far inside the 2e-2 gate.
    out_d = nc.dram_tensor("out", [S, D], BF16, kind="ExternalOutput").ap()

    # Linear views: partition p <-> rows [T*p, T*p+T), 8KB contiguous each.
    ctx_lin = ctx_d.rearrange("(p r) d -> p (r d)", p=P)
    main_lin = main_d.rearrange("(p r) d -> p (r d)", p=P)
    out_lin = out_d.rearrange("(p r) d -> p (r d)", p=P)

    with TileContext(nc) as tc:
      for _rep in range(repeats):
        with (
            tc.tile_pool(name="persist", bufs=1) as persist,
            tc.tile_pool(name="etile", bufs=4) as etile_pool,
            tc.tile_pool(name="small", bufs=4) as small,
            tc.tile_pool(name="tailp", bufs=4) as tailp,
            tc.tile_pool(name="psum_w", bufs=2, space="PSUM") as psum_w,
            tc.tile_pool(name="psum_acc", bufs=1, space="PSUM") as psum_acc,
        ):
            # ---- persistent SBUF tensors ----
            raw_c = persist.tile([P, T, P], F32)     # f32 staging (linear)
            raw_m = persist.tile([P, T, P], F32)
            ctx_h = persist.tile([P, T, P], F16)     # [c_in, j, d]
            main_h = persist.tile([P, T, P], F16)    # [m_in, j, d]
            ctxT = persist.tile([P, T, P], F16)      # [d, j, c_in]
            mainT = persist.tile([P, T, P], F16)     # [d, j, m_in]
            mainT2 = mainT.rearrange("p a b -> p (a b)")
            ident = persist.tile([P, P], F16)
            make_identity(nc, ident[:])

            # Warm the ACT exp table so the ~2.7us load overlaps the DMAs.
            warm = small.tile([P, 1], F32, tag="warm")
            nc.vector.memset(warm[:], 0.0)
            nc.scalar.activation(warm[:], warm[:], EXP)

            def q2(ap3, q):
                return ap3[:, q * QT:(q + 1) * QT].rearrange("p a b -> p (a b)")

            # ---- prologue loads: 256KB quarter-chunks (per-ring transfers
            # serialize at ~110GB/s, so what matters is putting the
            # earliest-needed chunks first on each of the 3 queues).
            def ldq(eng, raw, lin, q):
                eng.dma_start(q2(raw, q), lin[:, q * QW:(q + 1) * QW])

            # first wave = the exp(0)h0 gate {c q0, m q0, m q1}, one per
            # queue; second wave = {m q2, m q3} for h1; ctx rest trails.
            ldq(nc.sync, raw_c, ctx_lin, 0)
            ldq(nc.sync, raw_m, main_lin, 2)
            ldq(nc.scalar, raw_m, main_lin, 0)
            ldq(nc.gpsimd, raw_m, main_lin, 1)
            ldq(nc.gpsimd, raw_m, main_lin, 3)
            ldq(nc.gpsimd, raw_c, ctx_lin, 1)
            ldq(nc.gpsimd, raw_c, ctx_lin, 2)
            ldq(nc.gpsimd, raw_c, ctx_lin, 3)

            # ---- casts + d-major transposes ----
            acc = psum_acc.tile([P, T, P], F32)   # out accumulator [m_in, j, d]
            # Before the acc-init pass, the acc PSUM region doubles as the
            # transpose staging scratch (fp16 view) so psum_w's two slots
            # stay a clean mm1 double-buffer with h0/h1 slot parity.
            acc16 = acc.rearrange("p a b -> p (a b)").bitcast(F16)

            def cast_q(dst, src, q):
                nc.vector.tensor_copy(q2(dst, q), q2(src, q))

            def pe_transpose_batch(nat, dstT, ts, soff):
                nt = ts.stop - ts.start
                for i in range(nt):
                    nc.tensor.transpose(
                        acc16[:, soff + i * P:soff + (i + 1) * P],
                        nat[:, ts.start + i], ident[:],
                    )
                nc.vector.tensor_copy(
                    dstT[:, ts].rearrange("p a b -> p (a b)"),
                    acc16[:, soff:soff + nt * P],
                )

            # casts chase loads in arrival order; ordering rule: no
            # late-gated cast may precede an earlier-needed evacuation in
            # the in-order DVE queue (head-of-line blocking)
            cast_q(main_h, raw_m, 0)
            cast_q(ctx_h, raw_c, 0)
            pe_transpose_batch(main_h, mainT, slice(0, QT), 0 * QW)
            pe_transpose_batch(ctx_h, ctxT, slice(0, QT), 1 * QW)
            cast_q(main_h, raw_m, 1)
            pe_transpose_batch(main_h, mainT, slice(QT, 2 * QT), 2 * QW)
            cast_q(main_h, raw_m, 2)
            cast_q(main_h, raw_m, 3)
            pe_transpose_batch(main_h, mainT, slice(2 * QT, 3 * QT), 3 * QW)
            pe_transpose_batch(main_h, mainT, slice(3 * QT, 4 * QT), 4 * QW)
            # non-urgent ctx casts go to gpsimd (idle) so the DVE queue
            # stays clear for the mm1(0)-critical evacuations
            nc.gpsimd.tensor_copy(q2(ctx_h, 1), q2(raw_c, 1))
            nc.gpsimd.tensor_copy(q2(ctx_h, 2), q2(raw_c, 2))
            nc.gpsimd.tensor_copy(q2(ctx_h, 3), q2(raw_c, 3))

            # ---- matmul helpers ----
            w_tiles = {}

            def emit_mm1(ct, h):
                w = psum_w.tile([P, HALF], F32, tag="w")
                w_tiles[(ct, h)] = w
                for j in range(0, HALF, MMN):
                    nc.tensor.matmul(
                        w[:, j:j + MMN],
                        ctxT[:, ct],
                        mainT2[:, h * HALF + j: h * HALF + j + MMN],
                        start=True,
                        stop=True,
                    )

            def emit_ident(qs):
                # acc := +main (exact fp16 copy through the PE so PSUM
                # has_written bits are set for the accumulation group).
                # One N=512 matmul per 2KB PSUM bank, start=True zeroing it.
                for q in qs:
                    nc.tensor.matmul(
                        acc[:, q * QT:(q + 1) * QT].rearrange("p a b -> p (a b)"),
                        ident[:], q2(main_h, q),
                        start=True, stop=False,
                        skip_group_check=True,
                    )

            def emit_mm2(e_t, cs, mbs, stop):
                for mb in mbs:
                    nc.tensor.matmul(
                        acc[:, mb], e_t[:, mb * P:(mb + 1) * P], cs[:],
                        start=False, stop=stop,
                        skip_group_check=True,
                    )

            # prologue PE: first mm1 (slot parity: h0 -> slot0, h1 -> slot1)
            emit_mm1(0, 0)
            emit_mm1(0, 1)

            # ---- main loop (one-tile software pipeline) ----
            prev = None
            for ct in range(T):
                if ct < NQ - 1:
                    # ctxT quarters 1..3 ride the DMA xbar on the sync ring,
                    # which is idle during the loop; needed at tiles 4/8/12.
                    q = ct + 1
                    ts = slice(q * QT, (q + 1) * QT)
                    nc.sync.dma_start_transpose(ctxT[:, ts], q2(ctx_h, q))

                e_t = etile_pool.tile([P, S], BF16, tag="e")
                s_part = small.tile([P, 2], F32, tag="spart")
                nc.scalar.activation(e_t[:, 0:HALF], w_tiles[(ct, 0)][:], EXP)
                nc.vector.tensor_reduce(
                    s_part[:, 0:1], e_t[:, 0:HALF], axis=AX, op=ADD
                )
                if prev is not None:
                    emit_mm2(prev[0], prev[1], range(0, T // 2), stop=False)
                elif ct == 0:
                    emit_ident([0, 1])
                if ct + 1 < T:
                    emit_mm1(ct + 1, 0)
                nc.scalar.activation(
                    e_t[:, HALF:S], w_tiles[(ct, 1)][:], EXP,
                    accum_out=s_part[:, 1:2],
                )
                if prev is not None:
                    emit_mm2(prev[0], prev[1], range(T // 2, T), stop=False)
                elif ct == 0:
                    emit_ident([2, 3])
                if ct + 1 < T:
                    emit_mm1(ct + 1, 1)
                # stats: ctx_s = -context/S so acc accumulates main - weighted
                s_neg = small.tile([P, 1], F32, tag="ssum")
                nc.vector.tensor_scalar(
                    s_neg[:], s_part[:, 0:1], s_part[:, 1:2], -1.0, ADD, MULT
                )
                sinv = small.tile([P, 1], F32, tag="sinv")
                nc.vector.reciprocal(sinv[:], s_neg[:])
                ctx_s = small.tile([P, P], BF16, tag="ctxs")
                nc.vector.tensor_scalar_mul(ctx_s[:], ctx_h[:, ct], sinv[:])
                prev = (e_t, ctx_s)

            # ---- tail: ALL final mm2 blocks first (a PSUM read of acc
            # serializes later PE writes to it, so no mm2/copy interleave),
            # then bf16 evacs split DVE/ACT and one half-store per ring.
            e_l, cs_l = prev
            emit_mm2(e_l, cs_l, range(T), stop=True)
            # quarter evacs into SEPARATE tiles (same-tile writes from two
            # engines get serialized by the scheduler), DVE/ACT in
            # parallel, stores alternating rings
            for q in range(NQ):
                out_sb = tailp.tile([P, QT, P], BF16, tag="outsb")
                osb = out_sb.rearrange("p a b -> p (a b)")
                accq = acc[:, q * QT:(q + 1) * QT].rearrange("p a b -> p (a b)")
                if q % 2 == 0:
                    nc.vector.tensor_copy(osb, accq)
                    nc.sync.dma_start(out_lin[:, q * QW:(q + 1) * QW], osb)
                else:
                    nc.scalar.copy(osb, accq)
                    nc.scalar.dma_start(out_lin[:, q * QW:(q + 1) * QW], osb)

    nc.compile()
    return nc


_RUNNER_CACHE = {}


def _get_runner(S, D):
    """Compile once and return a reusable jitted SPMD runner.

    run_bass_kernel_spmd re-jits (and re-runs the NEFF compiler) on every
    call, so repeated kernel() invocations would each pay minutes of
    compile; this builds the bass_exec + shard_map executable one time.
    """
    key = (S, D)
    if key in _RUNNER_CACHE:
        return _RUNNER_CACHE[key]

    import jax
    import concourse.mybir as _mybir
    from concourse.bass2jax import (
        _bass_exec_p,
        install_neuronx_cc_hook,
        partition_id_tensor,
    )
    from jax.sharding import Mesh, PartitionSpec
    from jax.experimental.shard_map import shard_map

    install_neuronx_cc_hook()
    nc = build_nc(S, D)

    part_name = nc.partition_id_tensor.name if nc.partition_id_tensor else None
    in_names, out_names, out_avals, zero_outs = [], [], [], []
    for alloc in nc.m.functions[0].allocations:
        if not isinstance(alloc, _mybir.MemoryLocationSet):
            continue
        name = alloc.memorylocations[0].name
        if alloc.kind == "ExternalInput":
            if name == part_name:
                continue
            in_names.append(name)
        elif alloc.kind == "ExternalOutput":
            out_names.append(name)
            shape = tuple(alloc.tensor_shape)
            dtype = _mybir.dt.np(alloc.dtype)
            out_avals.append(jax.core.ShapedArray(shape, dtype))
            zero_outs.append(np.zeros(shape, dtype))

    all_in = in_names + out_names + ([part_name] if part_name else [])

    def _body(*args):
        operands = list(args)
        if part_name is not None:
            operands.append(partition_id_tensor())
        outs = _bass_exec_p.bind(
            *operands,
            out_avals=tuple(out_avals),
            in_names=tuple(all_in),
            out_names=tuple(out_names),
            lowering_input_output_aliases=(),
            sim_require_finite=True,
            sim_require_nnan=True,
            nc=nc,
        )
        return tuple(outs)

    devices = jax.devices()[:N_CORES]
    mesh = Mesh(np.asarray(devices), ("core",))
    nin = len(in_names) + len(out_names)
    sharded = jax.jit(
        shard_map(
            _body,
            mesh=mesh,
            in_specs=(PartitionSpec("core"),) * nin,
            out_specs=(PartitionSpec("core"),) * len(out_names),
            check_rep=False,
        ),
        keep_unused=True,
    )
    zeros_cat = [np.concatenate([z] * N_CORES, axis=0) for z in zero_outs]
    _RUNNER_CACHE[key] = (sharded, in_names, out_names, zeros_cat)
    return _RUNNER_CACHE[key]


def kernel(context: np.ndarray, main: np.ndarray) -> np.ndarray:
    B, S, D = context.shape
    assert main.shape == (B, S, D) and B == N_CORES
    sharded, in_names, out_names, zeros_cat = _get_runner(S, D)
    feed = {
        "context": np.ascontiguousarray(context, dtype=np.float32).reshape(B * S, D),
        "main": np.ascontiguousarray(main, dtype=np.float32).reshape(B * S, D),
    }
    args = [feed[n] for n in in_names] + zeros_cat
    outs = sharded(*args)
    out = np.asarray(outs[out_names.index("out")])
    return out.reshape(B, S, D).astype(np.float32)


# revision 28
# speedup vs baseline: 1.1617x; 1.1617x over previous
"""Trainium2 Bass kernel for AlignOnlySubLayer.

Per batch b:
    W[c,m]   = sum_d context[b,c,d] * main[b,m,d]
    A        = softmax(W, axis=m)
    out[m,d] = main[b,m,d] - sum_c A[c,m] * context[b,c,d]

Sharding: data-parallel over batch B=8 across the 8 NeuronCores (one batch
per core, no cross-core communication).

Kernel design (per core), measured on HW (exec ~64us vs 80.5us staged
baseline; the exec window is first-useful-instruction -> last-instruction
and includes ~10us of NRT semaphore-cleanup postamble that kernel code
cannot remove):
  - ACT is the algorithmic wall: 4M exps at 1 elem/lane/cycle @1.2GHz = 2
    x 1.11us half-row ACTIVATEs + ~0.24us READ_ACCUMULATOR per c-tile
    (~38us total).  Everything else is scheduled to keep that chain dense.
  - Linear DMA tiling: c/m tile j = rows {16p + j} (partition p = row//16)
    instead of row%128.  Per-partition DMA runs are then contiguous KBs
    (vs 512B for the row%128 tiling).  All compute is tile-local, so this
    is a pure relabeling - softmax rows stay intact, mm2's m-blocks match
    main's layout, and load/store/compute all agree.
  - DMA here is latency-bound (~350ns per descriptor per engine, ~60-110
    GB/s per transfer, ~190GB/s aggregate over the 3 issue queues), so
    loads are 256KB quarter-chunks spread over sync/scalar/gpsimd with
    the exp(0)h0 gate {ctx q0, main q0, main q1} first on each queue.
  - Both matmuls run fp16/bf16 (PE 1 col/cycle) with f32 PSUM.  mm1 fp16
    (mantissa), mm2 bf16 (E reaches ~e^70, needs f32 exponent range).
  - d-major operands via PE transpose-mode matmuls, staged through the
    NOT-YET-USED acc PSUM region (bitcast fp16 scratch) so psum_w's two
    slots stay a clean mm1 h0/h1 double-buffer; ctxT q1-3 ride the DMA
    xbar mid-loop (the xbar serializes against ALL outstanding DMAs -
    deadlock guard - so it must never sit in the load path).
  - acc is initialized with +main via 4 N=512 PE identity-matmuls
    (start=True, one per PSUM bank), and the softmax normalization is
    folded into ctx with a NEGATED scale (ctx_s = -context/S), so acc
    accumulates main-weighted directly and the tail is copy+store only.
  - Software pipeline, one c-tile lookahead: iteration ct emits
    exp(ct)h0, mm2(ct-1)[0..7], mm1(ct+1)h0, exp(ct)h1+accum,
    mm2(ct-1)[8..15], mm1(ct+1)h1, stats(ct).  PE per-period work
    (~1.9us warm) phases into ACT's ~2.36us period without stalling.
  - Row sums: h0 reduced on DVE (off ACT's critical path), h1 via ACT's
    fused accum_out; s = -(h0+h1) and reciprocal give the negated scale.
  - Tail: all 16 final mm2 blocks first (a PSUM read of acc serializes
    later PE writes to it), then quarter evacs into SEPARATE SBUF tiles
    (same-tile writes from two engines serialize) split DVE/ACT, bf16
    stores alternating both HWDGE rings.
  - Output is bf16 (cast to f32 on host): halves store traffic; adds
    ~2e-3 relative rounding on top of the ~1.8e-3 fp16-pipeline error,
    total ~2.4e-3 vs the 2e-2 gate.
  - Scheduling hazards learned on HW: the in-order DVE queue must never
    have a late-gated cast ahead of an earlier-needed evacuation
    (head-of-line blocking), and PE warm-up dummy matmuls are a net loss
    (they occupy the PE exactly when the transposes need it).
"""

import numpy as np

import concourse.bass as bass
import concourse.mybir as mybir
from concourse import bacc
from concourse.masks import make_identity
from concourse.tile import TileContext

P = 128
F32 = mybir.dt.float32
F16 = mybir.dt.float16
BF16 = mybir.dt.bfloat16
EXP = mybir.ActivationFunctionType.Exp
AX = mybir.AxisListType.X
ADD = mybir.AluOpType.add
MULT = mybir.AluOpType.mult
N_CORES = 8


def build_nc(S=2048, D=128, num_devices=N_CORES, repeats=1, precise=False):
    """Build the single-core Bass program (SPMD across cores)."""
    assert D == P and S % P == 0
    T = S // P            # tiles along c (and m); tile j = rows {T*p + j}
    NQ = 4                # quarters (load/store/transpose granularity)
    QT = T // NQ          # tiles per quarter
    QW = QT * P           # f32 elements per partition per quarter
    HALF = S // 2         # columns per mm1 psum half
    MMN = 512             # mm1 moving-operand chunk

    nc = bacc.Bacc(
        "TRN2",
        target_bir_lowering=False,
        debug=False,
        enable_asserts=False,
        num_devices=num_devices,
    )
    ctx_d = nc.dram_tensor("context", [S, D], F32, kind="ExternalInput").ap()
    main_d = nc.dram_tensor("main", [S, D], F32, kind="ExternalInput").ap()
    # bf16 output (cast to f32 on host): halves store bytes+descriptors,
    # costs ~2e-3 relative rounding on top of the ~1.8e-3 fp16-pipeline
    # error -- far inside the 2e-2 gate.
    out_d = nc.dram_tensor("out", [S, D], BF16, kind="ExternalOutput").ap()

    # Linear views: partition p <-> rows [T*p, T*p+T), 8KB contiguous each.
    ctx_lin = ctx_d.rearrange("(p r) d -> p (r d)", p=P)
    main_lin = main_d.rearrange("(p r) d -> p (r d)", p=P)
    out_lin = out_d.rearrange("(p r) d -> p (r d)", p=P)

    with TileContext(nc) as tc:
      for _rep in range(repeats):
        with (
            tc.tile_pool(name="persist", bufs=1) as persist,
            tc.tile_pool(name="etile", bufs=4) as etile_pool,
            tc.tile_pool(name="small", bufs=4) as small,
            tc.tile_pool(name="tailp", bufs=4) as tailp,
            tc.tile_pool(name="psum_w", bufs=2, space="PSUM") as psum_w,
            tc.tile_pool(name="psum_acc", bufs=1, space="PSUM") as psum_acc,
        ):
            # ---- persistent SBUF tensors ----
            raw_c = persist.tile([P, T, P], F32)     # f32 staging (linear)
            raw_m = persist.tile([P, T, P], F32)
            ctx_h = persist.tile([P, T, P], F16)     # [c_in, j, d]
            main_h = persist.tile([P, T, P], F16)    # [m_in, j, d]
            ctxT = persist.tile([P, T, P], F16)      # [d, j, c_in]
            mainT = persist.tile([P, T, P], F16)     # [d, j, m_in]
            mainT2 = mainT.rearrange("p a b -> p (a b)")
            ident = persist.tile([P, P], F16)
            make_identity(nc, ident[:])

            # Warm the ACT exp table so the ~2.7us load overlaps the DMAs.
            warm = small.tile([P, 1], F32, tag="warm")
            nc.vector.memset(warm[:], 0.0)
            nc.scalar.activation(warm[:], warm[:], EXP)

            def q2(ap3, q):
                return ap3[:, q * QT:(q + 1) * QT].rearrange("p a b -> p (a b)")

            # ---- prologue loads: 256KB quarter-chunks (per-ring transfers
            # serialize at ~110GB/s, so what matters is putting the
            # earliest-needed chunks first on each of the 3 queues).
            def ldq(eng, raw, lin, q):
                eng.dma_start(q2(raw, q), lin[:, q * QW:(q + 1) * QW])

            # first wave = the exp(0)h0 gate {c q0, m q0, m q1}, one per
            # queue; second wave = {m q2, m q3} for h1; ctx rest trails.
            ldq(nc.sync, raw_c, ctx_lin, 0)
            ldq(nc.sync, raw_m, main_lin, 2)
            ldq(nc.scalar, raw_m, main_lin, 0)
            ldq(nc.gpsimd, raw_m, main_lin, 1)
            ldq(nc.gpsimd, raw_m, main_lin, 3)
            ldq(nc.gpsimd, raw_c, ctx_lin, 1)
            ldq(nc.gpsimd, raw_c, ctx_lin, 2)
            ldq(nc.gpsimd, raw_c, ctx_lin, 3)

            # ---- casts + d-major transposes ----
            acc = psum_acc.tile([P, T, P], F32)   # out accumulator [m_in, j, d]
            # Before the acc-init pass, the acc PSUM region doubles as the
            # transpose staging scratch (fp16 view) so psum_w's two slots
            # stay a clean mm1 double-buffer with h0/h1 slot parity.
            acc16 = acc.rearrange("p a b -> p (a b)").bitcast(F16)

            def cast_q(dst, src, q):
                nc.vector.tensor_copy(q2(dst, q), q2(src, q))

            def pe_transpose_batch(nat, dstT, ts, soff):
                nt = ts.stop - ts.start
                for i in range(nt):
                    nc.tensor.transpose(
                        acc16[:, soff + i * P:soff + (i + 1) * P],
                        nat[:, ts.start + i], ident[:],
                    )
                nc.vector.tensor_copy(
                    dstT[:, ts].rearrange("p a b -> p (a b)"),
                    acc16[:, soff:soff + nt * P],
                )

            # casts chase loads in arrival order; ordering rule: no
            # late-gated cast may precede an earlier-needed evacuation in
            # the in-order DVE queue (head-of-line blocking)
            cast_q(main_h, raw_m, 0)
            cast_q(ctx_h, raw_c, 0)
            pe_transpose_batch(main_h, mainT, slice(0, QT), 0 * QW)
            pe_transpose_batch(ctx_h, ctxT, slice(0, QT), 1 * QW)
            cast_q(main_h, raw_m, 1)
            pe_transpose_batch(main_h, mainT, slice(QT, 2 * QT), 2 * QW)
            cast_q(main_h, raw_m, 2)
            cast_q(main_h, raw_m, 3)
            pe_transpose_batch(main_h, mainT, slice(2 * QT, 3 * QT), 3 * QW)
            pe_transpose_batch(main_h, mainT, slice(3 * QT, 4 * QT), 4 * QW)
            # non-urgent ctx casts go to gpsimd (idle) so the DVE queue
            # stays clear for the mm1(0)-critical evacuations
            nc.gpsimd.tensor_copy(q2(ctx_h, 1), q2(raw_c, 1))
            nc.gpsimd.tensor_copy(q2(ctx_h, 2), q2(raw_c, 2))
            nc.gpsimd.tensor_copy(q2(ctx_h, 3), q2(raw_c, 3))

            # ---- matmul helpers ----
            w_tiles = {}

            def emit_mm1(ct, h):
                w = psum_w.tile([P, HALF], F32, tag="w")
                w_tiles[(ct, h)] = w
                for j in range(0, HALF, MMN):
                    nc.tensor.matmul(
                        w[:, j:j + MMN],
                        ctxT[:, ct],
                        mainT2[:, h * HALF + j: h * HALF + j + MMN],
                        start=True,
                        stop=True,
                    )

            def emit_ident(qs):
                # acc := +main (exact fp16 copy through the PE so PSUM
                # has_written bits are set for the accumulation group).
                # One N=512 matmul per 2KB PSUM bank, start=True zeroing it.
                for q in qs:
                    nc.tensor.matmul(
                        acc[:, q * QT:(q + 1) * QT].rearrange("p a b -> p (a b)"),
                        ident[:], q2(main_h, q),
                        start=True, stop=False,
                        skip_group_check=True,
                    )

            def emit_mm2(e_t, cs, mbs, stop):
                for mb in mbs:
                    nc.tensor.matmul(
                        acc[:, mb], e_t[:, mb * P:(mb + 1) * P], cs[:],
                        start=False, stop=stop,
                        skip_group_check=True,
                    )

            # prologue PE: first mm1 (slot parity: h0 -> slot0, h1 -> slot1)
            emit_mm1(0, 0)
            emit_mm1(0, 1)

            # ---- main loop (one-tile software pipeline) ----
            prev = None
            for ct in range(T):
                if ct < NQ - 1:
                    # ctxT quarters 1..3 ride the DMA xbar on the sync ring,
                    # which is idle during the loop; needed at tiles 4/8/12.
                    q = ct + 1
                    ts = slice(q * QT, (q + 1) * QT)
                    nc.sync.dma_start_transpose(ctxT[:, ts], q2(ctx_h, q))

                e_t = etile_pool.tile([P, S], BF16, tag="e")
                s_part = small.tile([P, 2], F32, tag="spart")
                nc.scalar.activation(e_t[:, 0:HALF], w_tiles[(ct, 0)][:], EXP)
                nc.vector.tensor_reduce(
                    s_part[:, 0:1], e_t[:, 0:HALF], axis=AX, op=ADD
                )
                if prev is not None:
                    emit_mm2(prev[0], prev[1], range(0, T // 2), stop=False)
                elif ct == 0:
                    emit_ident([0, 1])
                if ct + 1 < T:
                    emit_mm1(ct + 1, 0)
                nc.scalar.activation(
                    e_t[:, HALF:S], w_tiles[(ct, 1)][:], EXP,
                    accum_out=s_part[:, 1:2],
                )
                if prev is not None:
                    emit_mm2(prev[0], prev[1], range(T // 2, T), stop=False)
                elif ct == 0:
                    emit_ident([2, 3])
                if ct + 1 < T:
                    emit_mm1(ct + 1, 1)
                # stats: ctx_s = -context/S so acc accumulates main - weighted
                s_neg = small.tile([P, 1], F32, tag="ssum")
                nc.vector.tensor_scalar(
                    s_neg[:], s_part[:, 0:1], s_part[:, 1:2], -1.0, ADD, MULT
                )
                sinv = small.tile([P, 1], F32, tag="sinv")
                nc.vector.reciprocal(sinv[:], s_neg[:])
                ctx_s = small.tile([P, P], BF16, tag="ctxs")
                nc.vector.tensor_scalar_mul(ctx_s[:], ctx_h[:, ct], sinv[:])
                prev = (e_t, ctx_s)

            # ---- tail: ALL final mm2 blocks first (a PSUM read of acc
            # serializes later PE writes to it, so no mm2/copy interleave),
            # then bf16 evacs split DVE/ACT and one half-store per ring.
            e_l, cs_l = prev
            emit_mm2(e_l, cs_l, range(T), stop=True)
            # quarter evacs into SEPARATE tiles (same-tile writes from two
            # engines get serialized by the scheduler), DVE/ACT in
            # parallel, stores alternating rings
            for q in range(NQ):
                out_sb = tailp.tile([P, QT, P], BF16, tag="outsb")
                osb = out_sb.rearrange("p a b -> p (a b)")
                accq = acc[:, q * QT:(q + 1) * QT].rearrange("p a b -> p (a b)")
                if q % 2 == 0:
                    nc.vector.tensor_copy(osb, accq)
                    nc.sync.dma_start(out_lin[:, q * QW:(q + 1) * QW], osb)
                else:
                    nc.scalar.copy(osb, accq)
                    nc.scalar.dma_start(out_lin[:, q * QW:(q + 1) * QW], osb)

    nc.compile()
    return nc


_RUNNER_CACHE = {}


def _get_runner(S, D):
    """Compile once and return a reusable jitted SPMD runner.

    run_bass_kernel_spmd re-jits (and re-runs the NEFF compiler) on every
    call, so repeated kernel() invocations would each pay minutes of
    compile; this builds the bass_exec + shard_map executable one time.
    """
    key = (S, D)
    if key in _RUNNER_CACHE:
        return _RUNNER_CACHE[key]

    import jax
    import concourse.mybir as _mybir
    from concourse.bass2jax import (
        _bass_exec_p,
        install_neuronx_cc_hook,
        partition_id_tensor,
    )
    from jax.sharding import Mesh, PartitionSpec
    from jax.experimental.shard_map import shard_map

    install_neuronx_cc_hook()
    nc = build_nc(S, D)

    part_name = nc.partition_id_tensor.name if nc.partition_id_tensor else None
    in_names, out_names, out_avals, zero_outs = [], [], [], []
    for alloc in nc.m.functions[0].allocations:
        if not isinstance(alloc, _mybir.MemoryLocationSet):
            continue
        name = alloc.memorylocations[0].name
        if alloc.kind == "ExternalInput":
            if name == part_name:
                continue
            in_names.append(name)
        elif alloc.kind == "ExternalOutput":
            out_names.append(name)
            shape = tuple(alloc.tensor_shape)
            dtype = _mybir.dt.np(alloc.dtype)
            out_avals.append(jax.core.ShapedArray(shape, dtype))
            zero_outs.append(np.zeros(shape, dtype))

    all_in = in_names + out_names + ([part_name] if part_name else [])

    def _body(*args):
        operands = list(args)
        if part_name is not None:
            operands.append(partition_id_tensor())
        outs = _bass_exec_p.bind(
            *operands,
            out_avals=tuple(out_avals),
            in_names=tuple(all_in),
            out_names=tuple(out_names),
            lowering_input_output_aliases=(),
            sim_require_finite=True,
            sim_require_nnan=True,
            nc=nc,
        )
        return tuple(outs)

    devices = jax.devices()[:N_CORES]
    mesh = Mesh(np.asarray(devices), ("core",))
    nin = len(in_names) + len(out_names)
    sharded = jax.jit(
        shard_map(
            _body,
            mesh=mesh,
            in_specs=(PartitionSpec("core"),) * nin,
            out_specs=(PartitionSpec("core"),) * len(out_names),
            check_rep=False,
        ),
        keep_unused=True,
    )
    zeros_cat = [np.concatenate([z] * N_CORES, axis=0) for z in zero_outs]
    _RUNNER_CACHE[key] = (sharded, in_names, out_names, zeros_cat)
    return _RUNNER_CACHE[key]


def kernel(context: np.ndarray, main: np.ndarray) -> np.ndarray:
    B, S, D = context.shape
    assert main.shape == (B, S, D) and B == N_CORES
    sharded, in_names, out_names, zeros_cat = _get_runner(S, D)
    feed = {
        "context": np.ascontiguousarray(context, dtype=np.float32).reshape(B * S, D),
        "main": np.ascontiguousarray(main, dtype=np.float32).reshape(B * S, D),
    }
    args = [feed[n] for n in in_names] + zeros_cat
    outs = sharded(*args)
    out = np.asarray(outs[out_names.index("out")])
    return out.reshape(B, S, D).astype(np.float32)
